# revision 65
# baseline (speedup 1.0000x reference)
# Multi-head attention (B=2, S=2048, D=1024, H=16, dh=64) on 8 TRN2 NeuronCores.
# Sharding: core = batch * 4 + head_group; each core handles one batch and 4 heads.
# Host prep: q/k/v transposed to feature-major bf16; weight slices transposed bf16.
#
# Braided pipeline, one period per 512-wide q-tile t:
#   proj-qk-hp0(t) -> attn(t,hp0,old k-tiles | filler: remaining proj chains
#   of t in 2-matmul chunks) -> attn(t,hp0,new diag tiles) -> evac ->
#   norm(t-1) -> attn(t,hp1 | filler: Wo units of t-1) -> evac
# so TensorE (matmuls), ScalarE (exp), DVE (evac/masks) and DMA overlap both
# within and across periods; x/keep tiles for t+1 prefetch during period t.
#
# Attention inner loop is software-pipelined (scores of tile n+1 are emitted
# before PV of tile n). scoresT = K^T@Q 2-head row-packed via tile_position
# (K=64 halves run concurrently); one exp per head-pair on ScalarE with the
# 1/8 scale fused; multiplicative keep-mask on diagonal tiles only; PV uses
# ones-augmented V (M=65) so softmax denominators fall out of the matmul;
# denominator rows stashed to SBUF on ScalarE. Normalization is deferred one
# period: fast reciprocal + K=1 tile-positioned broadcast matmuls + in-place
# DVE multiply reading PSUM directly. Row-parallel Wo with bf16 partial
# outputs DMA'd per 128-token block; host sums the 4 per-batch partials in
# f32.
#
# Hardware landmines learned on the way (sim passes, hw corrupts):
# partition-subrange DVE ops (e.g. reciprocal on [64:97]) and gpsimd
# partition_broadcast; DMA cannot read PSUM; only sync/scalar issue HWDGE.
import numpy as np
import ml_dtypes

import concourse.bass as bass
import concourse.tile as tile
from concourse import bacc, mybir
from concourse import bass_utils

B, S, D = 2, 2048, 1024
H, DH = 16, 64
NCORES = 8
GROUPS = 4            # head groups per batch (cores per batch)
HPG = 4               # heads per group
FPG = HPG * DH        # 256 features per group
SQ_T, SK_T = 512, 128
NSQ, NSK = S // SQ_T, S // SK_T
NCH = D // 128        # 8 contraction chunks of d_model
BF16 = ml_dtypes.bfloat16

_BUILT = {}


def _classify(mask):
    """Per-tile mask classification in scoresT space: tile (i, j) covers
    k in [i*128, (i+1)*128), q in [j*512, (j+1)*512)."""
    keep_t = (~np.asarray(mask, dtype=bool)).T  # [k, q], True = attend
    cls = {}
    ptiles = []
    for j in range(NSQ):
        for i in range(NSK):
            sub = keep_t[i * SK_T:(i + 1) * SK_T, j * SQ_T:(j + 1) * SQ_T]
            if not sub.any():
                cls[(i, j)] = "skip"
            elif sub.all():
                cls[(i, j)] = ("full", 0, SQ_T)
            else:
                # column bounding range with any unmasked entry
                cols = np.flatnonzero(sub.any(axis=0))
                cls[(i, j)] = (len(ptiles), int(cols[0]), int(cols[-1]) + 1)
                ptiles.append(np.ascontiguousarray(sub.astype(BF16)))
    return cls, ptiles


def _build(cls, n_ptiles):
    nc = bacc.Bacc("TRN2", target_bir_lowering=False, debug=False)
    dt = mybir.dt
    f32, bf = dt.float32, dt.bfloat16
    EXP = mybir.ActivationFunctionType.Exp

    xq = nc.dram_tensor("xqt", [D, S], bf, kind="ExternalInput").ap()
    xk = nc.dram_tensor("xkt", [D, S], bf, kind="ExternalInput").ap()
    xv = nc.dram_tensor("xvt", [D, S], bf, kind="ExternalInput").ap()
    wq = nc.dram_tensor("wqt", [D, FPG], bf, kind="ExternalInput").ap()
    wk = nc.dram_tensor("wkt", [D, FPG], bf, kind="ExternalInput").ap()
    wv = nc.dram_tensor("wvt", [D, FPG], bf, kind="ExternalInput").ap()
    wo = nc.dram_tensor("wot", [FPG, D], bf, kind="ExternalInput").ap()
    kp = nc.dram_tensor("keep", [max(n_ptiles, 1) * SK_T, SQ_T], bf,
                        kind="ExternalInput").ap()
    out = nc.dram_tensor("out", [S, D], bf, kind="ExternalOutput").ap()

    xq_v = xq.rearrange("(c p) s -> p c s", p=128)
    xk_v = xk.rearrange("(c p) s -> p c s", p=128)
    xv_v = xv.rearrange("(c p) s -> p c s", p=128)
    wq_v = wq.rearrange("(c p) f -> p c f", p=128)
    wk_v = wk.rearrange("(c p) f -> p c f", p=128)
    wv_v = wv.rearrange("(c p) f -> p c f", p=128)
    wo_v = wo.rearrange("(c p) o -> p c o", p=128)
    kp_v = kp.rearrange("(n p) s -> p n s", p=128)
    out_v = out.rearrange("(r p) o -> r p o", p=128)

    with tile.TileContext(nc) as tc:
        with (
            tc.tile_pool(name="consts", bufs=1) as consts,
            tc.tile_pool(name="x", bufs=2) as xpool,
            tc.tile_pool(name="sc", bufs=2, space="PSUM") as sc_ps,
            tc.tile_pool(name="pv", bufs=2, space="PSUM") as pv_ps,
            tc.tile_pool(name="aux", bufs=2, space="PSUM") as aux_ps,
            tc.tile_pool(name="work", bufs=4) as work,
            tc.tile_pool(name="ost", bufs=8) as ostpool,
            tc.tile_pool(name="probs", bufs=8) as prpool,
        ):
            wq_sb = consts.tile([128, NCH, FPG], bf)
            wk_sb = consts.tile([128, NCH, FPG], bf)
            wv_sb = consts.tile([128, NCH, FPG], bf)
            wo_sb = consts.tile([128, FPG // 128, D], bf)
            keep_sb = consts.tile([128, max(n_ptiles, 1), SQ_T], bf)
            ones_sb = consts.tile([128, 128], bf)
            qh_sb = consts.tile([128, 2, S], bf)
            kh_sb = consts.tile([128, 2, S], bf)
            vh_sb = consts.tile([128, NSK, HPG, DH + 1], bf)
            att_sb = consts.tile([128, 2, S], bf)
            # denominators per (j, hp): head-even at partition 64*hp,
            # head-odd at partition 64*hp+32 (legal DVE write starts)
            l4_sb = consts.tile([128, NSQ, SQ_T], f32)

            nc.vector.memset(ones_sb[:], 1.0)
            # only the ones column of V needs initialization
            nc.vector.memset(vh_sb[:, :, :, DH:DH + 1], 1.0)
            nc.vector.memset(l4_sb[:], 1.0)  # unused partitions -> r = 1

            # per-j contiguous partial-tile ranges in the packed keep tensor
            kranges = []
            lo = 0
            for j in range(NSQ):
                hi = lo
                for i in range(NSK):
                    cj = cls[(i, j)]
                    if cj != "skip" and cj[0] != "full":
                        hi = cj[0] + 1
                kranges.append((lo, max(hi, lo)))
                lo = max(hi, lo)

            x_tiles = {}

            def prefetch_x(t, split, eng):
                sl = bass.ts(t, SQ_T)
                xq_t = xpool.tile([128, NCH, SQ_T], bf, tag="xq")
                xk_t = xpool.tile([128, NCH, SQ_T], bf, tag="xk")
                xv_t = xpool.tile([128, NCH, SQ_T], bf, tag="xv")
                if split:
                    # t=0: xq/xk on the scalar queue; xv is issued by the
                    # caller on sync AFTER the weights (xk then arrives a
                    # transfer earlier for the kh projection)
                    eng.dma_start(xq_t[:, 0:1, :], xq_v[:, 0:1, sl])
                    eng.dma_start(xq_t[:, 1:4, :], xq_v[:, 1:4, sl])
                    eng.dma_start(xq_t[:, 4:8, :], xq_v[:, 4:8, sl])
                    eng.dma_start(xk_t[:, 0:4, :], xk_v[:, 0:4, sl])
                    eng.dma_start(xk_t[:, 4:8, :], xk_v[:, 4:8, sl])
                else:
                    eng.dma_start(xq_t[:], xq_v[:, :, sl])
                    eng.dma_start(xk_t[:], xk_v[:, :, sl])
                    eng.dma_start(xv_t[:], xv_v[:, :, sl])
                x_tiles[t] = (xq_t, xk_t, xv_t)

            def proj_hp(wsb, xin, hout, hp, sl):
                ps = aux_ps.tile([128, SQ_T], f32, tag="aux")
                hsl = bass.ts(hp, 128)
                for c in range(NCH):
                    nc.tensor.matmul(ps[:], wsb[:, c, hsl], xin[:, c, :],
                                     start=(c == 0), stop=(c == NCH - 1))
                nc.vector.tensor_copy(hout[:, hp, sl], ps[:])

            def phase_a1(t):
                sl = bass.ts(t, SQ_T)
                if t == 0:
                    nc.sync.dma_start(wq_sb[:, 0:1, :], wq_v[:, 0:1, :])
                    prefetch_x(0, True, nc.scalar)
                    nc.sync.dma_start(wq_sb[:, 1:4, :], wq_v[:, 1:4, :])
                    nc.sync.dma_start(wq_sb[:, 4:8, :], wq_v[:, 4:8, :])
                    nc.sync.dma_start(wk_sb[:], wk_v[:])
                    nc.sync.dma_start(x_tiles[0][2][:],
                                      xv_v[:, :, bass.ts(0, SQ_T)])
                    nc.scalar.dma_start(wv_sb[:], wv_v[:])
                    klo, khi = kranges[0]
                    if khi > klo:
                        # first mask tile rides sync (needed earliest by the
                        # diagonal attention of t=0); the rest stays on scalar
                        nc.sync.dma_start(keep_sb[:, klo:klo + 1, :],
                                          kp_v[:, klo:klo + 1, :])
                        if khi > klo + 1:
                            nc.scalar.dma_start(keep_sb[:, klo + 1:khi, :],
                                                kp_v[:, klo + 1:khi, :])
                xq_t, xk_t, _ = x_tiles[t]
                proj_hp(wq_sb, xq_t, qh_sb, 0, sl)
                proj_hp(wk_sb, xk_t, kh_sb, 0, sl)

            def proj_hp_chunks(wsb, xin, hout, hp, sl):
                # one projection chain split into 2-matmul chunks (the psum
                # accumulation group stays open across chunks) so it can be
                # dribbled between attention tiles without starving the exp
                # stream
                hsl = bass.ts(hp, 128)
                state = {}

                def chunk(c0):
                    if c0 == 0:
                        state["ps"] = aux_ps.tile([128, SQ_T], f32,
                                                  tag="aux", name="pchunk")
                    ps = state["ps"]
                    for c in (c0, c0 + 1):
                        nc.tensor.matmul(ps[:], wsb[:, c, hsl], xin[:, c, :],
                                         start=(c == 0),
                                         stop=(c == NCH - 1))
                    if c0 == NCH - 2:
                        nc.vector.tensor_copy(hout[:, hp, sl], ps[:])
                return [lambda c0=c0: chunk(c0) for c0 in range(0, NCH, 2)]

            def vproj_chunks(t, s4):
                i = t * (SQ_T // SK_T) + s4
                state = {}

                def chunk(c0):
                    xv_t = x_tiles[t][2]
                    if c0 == 0:
                        state["ps"] = aux_ps.tile([128, SQ_T], f32,
                                                  tag="aux", name="vchunk")
                    psv = state["ps"]
                    for c in (c0, c0 + 1):
                        nc.tensor.matmul(psv[:, 0:FPG],
                                         xv_t[:, c, bass.ts(s4, SK_T)],
                                         wv_sb[:, c, :],
                                         start=(c == 0),
                                         stop=(c == NCH - 1))
                    if c0 == NCH - 2:
                        nc.vector.tensor_copy(
                            vh_sb[:, i, :, 0:DH],
                            psv[:, 0:FPG].rearrange("p (h d) -> p h d",
                                                    h=HPG))
                return [lambda c0=c0: chunk(c0) for c0 in range(0, NCH, 2)]

            def a2_dmas(t):
                if t + 1 < NSQ:
                    klo, khi = kranges[t + 1]
                    if khi > klo:
                        nc.sync.dma_start(keep_sb[:, klo:khi, :],
                                          kp_v[:, klo:khi, :])
                    prefetch_x(t + 1, False, nc.sync)
                if t == 1:
                    nc.sync.dma_start(wo_sb[:], wo_v[:])

            def phase_a2_thunks(t):
                sl = bass.ts(t, SQ_T)
                xq_t, xk_t, _ = x_tiles[t]
                th = []
                th += proj_hp_chunks(wq_sb, xq_t, qh_sb, 1, sl)
                th += proj_hp_chunks(wk_sb, xk_t, kh_sb, 1, sl)
                for s4 in range(SQ_T // SK_T):
                    th += vproj_chunks(t, s4)
                # next-period prefetches last: they must not steal DMA-pool
                # bandwidth from this period's own xv/wv loads
                th.append(lambda: a2_dmas(t))
                return th

            def make_filler(thunks, delay=0, per_call=1):
                state = {"u": 0}

                def filler(k):
                    if k < delay:
                        return
                    for _ in range(per_call):
                        if state["u"] < len(thunks):
                            thunks[state["u"]]()
                            state["u"] += 1

                def flush():
                    while state["u"] < len(thunks):
                        thunks[state["u"]]()
                        state["u"] += 1
                return filler, flush

            def attn_range(j, hp, pv0, pv1, kept, n0, n1, filler=None):
                # software-pipelined: scores/exp of tile n+1 are emitted
                # before the PV matmuls of tile n, so the tensor queue never
                # head-of-line blocks on the exp chain. `filler` emits
                # unrelated tensor work (wo of the previous q-tile) spread
                # between iterations.
                pend = None

                def flush_pend(last):
                    if pend is None:
                        return
                    n, c0, c1, pr = pend
                    nc.tensor.matmul(pv0[:, c0:c1],
                                     vh_sb[:, kept[n], 2 * hp + 0, :],
                                     pr[:, 0, c0:c1], start=(n == 0),
                                     stop=last)
                    nc.tensor.matmul(pv1[:, c0:c1],
                                     vh_sb[:, kept[n], 2 * hp + 1, :],
                                     pr[:, 1, c0:c1], start=(n == 0),
                                     stop=last)

                for n in range(n0, n1):
                    i = kept[n]
                    isl = bass.ts(i, SK_T)
                    c, c0, c1 = cls[(i, j)]
                    if n == 0:
                        c0, c1 = 0, SQ_T  # first tile must cover the bank
                    qsl = bass.ds(j * SQ_T + c0, c1 - c0)
                    sc = sc_ps.tile([128, 2, SQ_T], f32, tag="sc")
                    nc.tensor.matmul(sc[:, 0, c0:c1], kh_sb[0:64, hp, isl],
                                     qh_sb[0:64, hp, qsl], start=True,
                                     stop=True, tile_position=(0, 0))
                    nc.tensor.matmul(sc[:, 1, c0:c1], kh_sb[64:128, hp, isl],
                                     qh_sb[64:128, hp, qsl], start=True,
                                     stop=True, tile_position=(64, 0))
                    pr = prpool.tile([128, 2, SQ_T], bf, tag="probs")
                    nc.scalar.activation(pr[:, :, c0:c1], sc[:, :, c0:c1],
                                         EXP, scale=0.125)
                    if c != "full":
                        nc.vector.tensor_mul(pr[:, 0, c0:c1],
                                             pr[:, 0, c0:c1],
                                             keep_sb[:, c, c0:c1])
                        nc.vector.tensor_mul(pr[:, 1, c0:c1],
                                             pr[:, 1, c0:c1],
                                             keep_sb[:, c, c0:c1])
                    flush_pend(False)
                    if filler is not None:
                        filler(n - n0)
                    pend = (n, c0, c1, pr)
                flush_pend(n1 == len(kept))

            CPY = mybir.ActivationFunctionType.Copy

            def attn_evac(j, hp, pv0, pv1):
                jsl = bass.ts(j, SQ_T)
                p0 = 64 * hp
                nc.vector.tensor_copy(att_sb[0:64, hp, jsl], pv0[0:DH, :])
                nc.vector.tensor_copy(att_sb[64:128, hp, jsl], pv1[0:DH, :])
                # denominator rows go out on the (idle) scalar engine so the
                # vector queue is not serialized at period boundaries
                nc.scalar.activation(l4_sb[p0:p0 + 1, j, :],
                                     pv0[DH:DH + 1, :], CPY)
                nc.scalar.activation(l4_sb[p0 + 32:p0 + 33, j, :],
                                     pv1[DH:DH + 1, :], CPY)

            def norm_prep(j, half=None):
                # always full-partition ops (DVE cost depends only on free
                # size; partition-subrange DVE ops misbehave on hardware).
                # When called for one hp the other hp's denominator rows just
                # hold their memset value.
                r4f = work.tile([128, SQ_T], f32, tag="r4f")
                r4 = work.tile([128, SQ_T], bf, tag="r4")
                nc.vector.reciprocal_approx_fast(r4f[:], l4_sb[:, j, :])
                nc.vector.tensor_copy(r4[:], r4f[:])
                return r4

            def norm(j, hp, r4):
                jsl = bass.ts(j, SQ_T)
                p0 = 64 * hp
                p1 = p0 + 32
                rb = aux_ps.tile([128, SQ_T], f32, tag="aux")
                nc.tensor.matmul(rb[0:64, :], ones_sb[p0:p0 + 1, 0:64],
                                 r4[p0:p0 + 1, :], start=True, stop=True,
                                 tile_position=(p0, 0))
                nc.tensor.matmul(rb[64:128, :], ones_sb[p1:p1 + 1, 64:128],
                                 r4[p1:p1 + 1, :], start=True, stop=True,
                                 tile_position=(p1, 64))
                nc.vector.tensor_mul(att_sb[:, hp, jsl], att_sb[:, hp, jsl],
                                     rb[:])

            def wo_unit(j, u, alternate=False):
                t4, o = u // 2, u % 2
                r_ = j * (SQ_T // 128) + t4
                tsl = bass.ds(j * SQ_T + t4 * 128, 128)
                po = aux_ps.tile([128, SQ_T], f32, tag="aux")
                for hp in range(2):
                    nc.tensor.matmul(po[:], att_sb[:, hp, tsl],
                                     wo_sb[:, hp, bass.ts(o, 512)],
                                     start=(hp == 0), stop=(hp == 1))
                ost = ostpool.tile([128, 512], bf, tag="ost")
                if alternate and u == 7:
                    # final unit: halve the exposed cast+DMA drain by
                    # splitting across both engine pairs
                    nc.vector.tensor_copy(ost[:, 0:256], po[:, 0:256])
                    nc.scalar.activation(ost[:, 256:512], po[:, 256:512],
                                         CPY)
                    nc.sync.dma_start(
                        out_v[r_, :, bass.ds(o * 512, 256)], ost[:, 0:256])
                    nc.scalar.dma_start(
                        out_v[r_, :, bass.ds(o * 512 + 256, 256)],
                        ost[:, 256:512])
                elif alternate and u % 2 == 1:
                    nc.scalar.activation(ost[:], po[:], CPY)
                    nc.scalar.dma_start(out_v[r_, :, bass.ts(o, 512)],
                                        ost[:])
                else:
                    nc.vector.tensor_copy(ost[:], po[:])
                    nc.sync.dma_start(out_v[r_, :, bass.ts(o, 512)], ost[:])

            def wo_proj(j, alternate=False):
                for u in range(8):
                    wo_unit(j, u, alternate=alternate)

            # ---- braided main loop ----
            for t in range(NSQ):
                phase_a1(t)
                kept = [i for i in range(NSK) if cls[(i, t)] != "skip"]
                nold = sum(1 for i in kept if i < 4 * t)
                pv00 = pv_ps.tile([DH + 1, SQ_T], f32, tag="pv")
                pv01 = pv_ps.tile([DH + 1, SQ_T], f32, tag="pv")
                # hp0 attention over old k-tiles with the remaining
                # projection chains spread between its tiles: the exp stream
                # paces attention while the PE stays saturated on projections
                a2f, a2_flush = make_filler(phase_a2_thunks(t), per_call=2)
                attn_range(t, 0, pv00, pv01, kept, 0, nold, filler=a2f)
                a2_flush()
                attn_range(t, 0, pv00, pv01, kept, nold, len(kept))
                attn_evac(t, 0, pv00, pv01)
                if t >= 1:
                    # deferred normalization + output projection of the
                    # previous q-tile: denominators were evacuated half a
                    # period ago, so the rb matmuls never stall the tensor
                    # queue; wo units are spread between the hp1 attention
                    # tiles so the exp stream starts immediately
                    r4 = norm_prep(t - 1)
                    norm(t - 1, 0, r4)
                    norm(t - 1, 1, r4)
                    wof, wo_flush = make_filler(
                        [lambda u=u: wo_unit(t - 1, u) for u in range(8)],
                        delay=2)
                else:
                    wof, wo_flush = None, lambda: None
                if t == NSQ - 1 and wo_flush is not None:
                    # normalize the last tile's hp0 mid-period (appended to
                    # the wo filler so its rb matmul lands between attention
                    # tiles): denominators are already in l4 (hp1 rows read
                    # their memset value and are unused), so only hp1's
                    # chain is exposed in the tail
                    wof, wo_flush = make_filler(
                        [lambda u=u: wo_unit(t - 1, u) for u in range(8)]
                        + [lambda: norm(t, 0, norm_prep(t))], delay=2)
                pv10 = pv_ps.tile([DH + 1, SQ_T], f32, tag="pv")
                pv11 = pv_ps.tile([DH + 1, SQ_T], f32, tag="pv")
                attn_range(t, 1, pv10, pv11, kept, 0, len(kept), filler=wof)
                wo_flush()
                attn_evac(t, 1, pv10, pv11)
            r4t = norm_prep(NSQ - 1)
            jsl3 = bass.ts(NSQ - 1, SQ_T)
            rbt = aux_ps.tile([128, SQ_T], f32, tag="aux")
            nc.tensor.matmul(rbt[0:64, :], ones_sb[64:65, 0:64],
                             r4t[64:65, :], start=True, stop=True,
                             tile_position=(64, 0))
            nc.tensor.matmul(rbt[64:128, :], ones_sb[96:97, 64:128],
                             r4t[96:97, :], start=True, stop=True,
                             tile_position=(96, 64))
            # per-128-token multiply so the first wo units start sooner
            for t4 in range(4):
                csl = bass.ds((NSQ - 1) * SQ_T + t4 * 128, 128)
                nc.vector.tensor_mul(att_sb[:, 1, csl], att_sb[:, 1, csl],
                                     rbt[:, bass.ts(t4, 128)])
                wo_unit(NSQ - 1, 2 * t4, alternate=True)
                wo_unit(NSQ - 1, 2 * t4 + 1, alternate=True)

    nc.compile()
    return nc


def _get_nc(mask):
    key = hash(np.asarray(mask, dtype=bool).tobytes())
    if key not in _BUILT:
        cls, ptiles = _classify(mask)
        _BUILT[key] = (_build(cls, len(ptiles)), cls, ptiles)
    return _BUILT[key]


def _kernel_impl(q, k, v, attn_mask, Wq, Wk, Wv, Wo, trace=False):
    q = np.asarray(q, dtype=np.float32)
    k = np.asarray(k, dtype=np.float32)
    v = np.asarray(v, dtype=np.float32)
    nc, cls, ptiles = _get_nc(attn_mask)

    if ptiles:
        keep_packed = np.concatenate(ptiles, axis=0)
    else:
        keep_packed = np.zeros((SK_T, SQ_T), dtype=BF16)

    xt = {}
    for b in range(B):
        xt[("q", b)] = np.ascontiguousarray(q[b].T.astype(BF16))
        xt[("k", b)] = np.ascontiguousarray(k[b].T.astype(BF16))
        xt[("v", b)] = np.ascontiguousarray(v[b].T.astype(BF16))
    wslices = {}
    for g in range(GROUPS):
        fsl = slice(g * FPG, (g + 1) * FPG)
        wslices[("wq", g)] = np.ascontiguousarray(Wq[fsl, :].T.astype(BF16))
        wslices[("wk", g)] = np.ascontiguousarray(Wk[fsl, :].T.astype(BF16))
        wslices[("wv", g)] = np.ascontiguousarray(Wv[fsl, :].T.astype(BF16))
        wslices[("wo", g)] = np.ascontiguousarray(Wo[:, fsl].T.astype(BF16))

    in_maps = []
    for core in range(NCORES):
        b, g = core // GROUPS, core % GROUPS
        in_maps.append({
            "xqt": xt[("q", b)], "xkt": xt[("k", b)], "xvt": xt[("v", b)],
            "wqt": wslices[("wq", g)], "wkt": wslices[("wk", g)],
            "wvt": wslices[("wv", g)], "wot": wslices[("wo", g)],
            "keep": keep_packed,
        })

    res = bass_utils.run_bass_kernel_spmd(
        nc, in_maps, core_ids=list(range(NCORES)), trace=trace)

    out = np.zeros((B, S, D), dtype=np.float32)
    for core in range(NCORES):
        out[core // GROUPS] += res.results[core]["out"].astype(np.float32)
    return out, res


def kernel(q, k, v, attn_mask, Wq, Wk, Wv, Wo):
    out, _ = _kernel_impl(q, k, v, attn_mask, Wq, Wk, Wv, Wo)
    return out
